# revision 10
# baseline (speedup 1.0000x reference)
"""Trainium2 Bass kernel for nn_Attention_54245436948569.

Full multi-head attention (qkv proj + interleaved RoPE + softmax attention +
out proj) for B=2, N=2048, D=1024, H=16, DH=64, sharded over 8 NeuronCores as
(batch x head-group): core c handles batch c//4 and heads [4*(c%4), 4*(c%4)+4).

Per-core kernel computes a row-parallel partial of the out-projection
([2048, 1024] fp32); the host sums the 4 partials per batch and adds b_out
(the unshard step for row-parallel tensor parallelism).

Matmuls run in bf16 (fp32 PSUM accumulation); softmax runs in fp32 on the
scalar engine with the 1/sqrt(DH) scale folded into exp. The softmax
denominator rides the AV matmul as a ones-column appended to V.
"""

import os
import numpy as np
import ml_dtypes

B, N, D = 2, 2048, 1024
H, DH = 16, 64
THETA = 10000.0

BF = ml_dtypes.bfloat16

_CACHE = {}


def _build():
    import concourse.bass as bass
    import concourse.mybir as mybir
    import concourse.tile as tile
    from concourse import bacc

    FP32 = mybir.dt.float32
    BF16 = mybir.dt.bfloat16
    AF = mybir.ActivationFunctionType
    MUL = mybir.AluOpType.mult
    ADD = mybir.AluOpType.add

    from concourse.compiler_utils import get_compiler_flags, set_compiler_flags
    set_compiler_flags([f.replace("--enable-ldw-opt=false", "--enable-ldw-opt=true")
                        for f in get_compiler_flags()])

    nc = bacc.Bacc(None, target_bir_lowering=False)

    NT = N // 512            # 4 token 512-blocks
    KT_D = D // 128          # 8 contraction tiles for qkv
    KT_N = N // 128          # 16 k-token tiles for attention

    with tile.TileContext(nc) as tc:
        with tc.tile_pool(name="dram", bufs=1, space="DRAM") as dram:
            xT_d = dram.tile([KT_D, 128, N], BF16, kind="ExternalInput", name="xT", uniquify=False)
            wqk_d = dram.tile([KT_D, 128, 512], BF16, kind="ExternalInput", name="wqk", uniquify=False)
            wv_d = dram.tile([KT_D, 128, 256], BF16, kind="ExternalInput", name="wv", uniquify=False)
            wo_d = dram.tile([2, 128, 1024], BF16, kind="ExternalInput", name="wo", uniquify=False)
            cos_d = dram.tile([128, N], BF16, kind="ExternalInput", name="cos2", uniquify=False)
            sin_d = dram.tile([128, N], BF16, kind="ExternalInput", name="sin2n", uniquify=False)
            out_d = dram.tile([KT_N, 128, D], FP32, kind="ExternalOutput", name="out", uniquify=False)

            from contextlib import ExitStack
            ctx = stack = ExitStack()
            stack.__enter__()
            const = ctx.enter_context(tc.tile_pool(name="const", bufs=1))
            ropep = ctx.enter_context(tc.tile_pool(name="ropep", bufs=3))
            attnp = ctx.enter_context(tc.tile_pool(name="attnp", bufs=4))
            stkp = ctx.enter_context(tc.tile_pool(name="stkp", bufs=3))
            normp = ctx.enter_context(tc.tile_pool(name="normp", bufs=3))
            outp = ctx.enter_context(tc.tile_pool(name="outp", bufs=3))
            # PSUM budget (8 banks): misc 2 x 1 bank, scores 2 x 2 banks, av 2 x 1 bank
            ps_misc = ctx.enter_context(tc.tile_pool(name="ps_misc", bufs=2, space="PSUM"))
            ps_sc = ctx.enter_context(tc.tile_pool(name="ps_sc", bufs=2, space="PSUM"))
            ps_av = ctx.enter_context(tc.tile_pool(name="ps_av", bufs=2, space="PSUM"))

            # ---- persistent SBUF tensors ----
            xT = const.tile([128, KT_D, N], BF16)
            wqk = const.tile([128, KT_D, 512], BF16)
            wv = const.tile([128, KT_D, 256], BF16)
            wo = const.tile([128, 2, 1024], BF16)
            cos2 = const.tile([128, N], BF16)
            sin2n = const.tile([128, N], BF16)
            q2t = [const.tile([128, N], BF16, name=f"q2_{p}") for p in range(2)]
            k2t = [const.tile([128, N], BF16, name=f"k2_{p}") for p in range(2)]
            v_t = [const.tile([128, 4, 65], BF16, name=f"v_{tt}") for tt in range(KT_N)]
            ones1 = const.tile([128, 64], FP32)          # K=1 broadcast weights (row 64 used)

            with nc.named_scope("load"):
                nc.sync.dma_start(out=xT[:, 0:4, :], in_=xT_d[0:4].rearrange("k p n -> p k n"))
                nc.gpsimd.dma_start(out=xT[:, 4:8, :], in_=xT_d[4:8].rearrange("k p n -> p k n"))
                nc.scalar.dma_start(out=wqk[:], in_=wqk_d.rearrange("k p n -> p k n"))
                nc.scalar.dma_start(out=cos2[:], in_=cos_d[:])
                nc.scalar.dma_start(out=sin2n[:], in_=sin_d[:])
                nc.gpsimd.dma_start(out=wv[:], in_=wv_d.rearrange("k p n -> p k n"))
                nc.scalar.dma_start(out=wo[:], in_=wo_d.rearrange("k p n -> p k n"))
                nc.vector.memset(ones1[:], 1.0)
                for tt in range(KT_N):
                    nc.vector.memset(v_t[tt][:, :, 64:65], 1.0)

            pair_mask = []
            for i in range(16):
                pair_mask += [2 * i + 1, 2 * i]

            # ---- Phase 1: qkv projection + RoPE ----
            def qk_proj(m):
                dest = (q2t if m < 2 else k2t)[m % 2]
                for nt in range(NT):
                    pqk = ps_misc.tile([128, 512], FP32, tag="misc", name="pqk")
                    for kt in range(KT_D):
                        nc.tensor.matmul(
                            pqk[:],
                            wqk[:, kt, m * 128:(m + 1) * 128],
                            xT[:, kt, nt * 512:(nt + 1) * 512],
                            start=(kt == 0), stop=(kt == KT_D - 1),
                        )
                    # evict on ScalarE (frees PSUM with a single reader), then
                    # RoPE on DVE in bf16: rot = q*cos2 + pairswap(q)*sin2n
                    ts = slice(nt * 512, (nt + 1) * 512)
                    qraw = ropep.tile([128, 512], BF16, name="qraw")
                    if m >= 2:
                        nc.scalar.activation(qraw[:], pqk[:], AF.Copy)
                    else:
                        nc.vector.tensor_copy(qraw[:], pqk[:])
                    qcos = ropep.tile([128, 512], BF16, name="qcos")
                    qsw = ropep.tile([128, 512], BF16, name="qsw")
                    tmp = ropep.tile([128, 512], BF16, name="tmp")
                    nc.vector.tensor_tensor(out=qcos[:], in0=qraw[:], in1=cos2[:, ts], op=MUL)
                    nc.vector.stream_shuffle(qsw[:], qraw[:], pair_mask)
                    nc.vector.tensor_tensor(out=tmp[:], in0=qsw[:], in1=sin2n[:, ts], op=MUL)
                    nc.vector.tensor_tensor(out=dest[:, ts], in0=qcos[:], in1=tmp[:], op=ADD)

            with nc.named_scope("qkv"):
                # k-pair0 + q-pair0 first so attention on pair0 can start while
                # the rest of qkv (pair1, v) streams in behind it.
                qk_proj(2)
                qk_proj(0)
                qk_proj(3)
                qk_proj(1)
                for tt in range(KT_N):
                    pv = ps_misc.tile([128, 512], FP32, tag="misc", name="pv")
                    for kt in range(KT_D):
                        nc.tensor.matmul(
                            pv[:, 0:256],
                            xT[:, kt, tt * 128:(tt + 1) * 128],
                            wv[:, kt, :],
                            start=(kt == 0), stop=(kt == KT_D - 1),
                        )
                    # scatter 4 head-slices into v_t[tt][:, h, 0:64]
                    nc.vector.tensor_copy(v_t[tt][:, :, 0:64], pv[:, 0:256].rearrange("p (h d) -> p h d", d=64))

            # ---- Phase 2: attention + out-proj per q-block ----
            SCALE = 1.0 / float(np.sqrt(DH))
            for qb in range(NT):
                qs_ = slice(qb * 512, (qb + 1) * 512)
                stacked = []
                for p in range(2):
                    with nc.named_scope(f"scores_p{p}_qb{qb}"):
                        pav_a = ps_av.tile([128, 512], FP32, tag="pav", name="pav_a")
                        pav_b = ps_av.tile([128, 512], FP32, tag="pav", name="pav_b")
                        for kt in range(KT_N):
                            pg = ps_sc.tile([128, 2, 512], FP32, tag="pg", name="pg")
                            attnT = attnp.tile([128, 2, 512], BF16, tag="attnT", name="attnT")
                            ks = slice(kt * 128, (kt + 1) * 128)
                            nc.tensor.matmul(
                                pg[:, 0, :], k2t[p][0:64, ks], q2t[p][0:64, qs_],
                                start=True, stop=True, tile_position=(0, 0),
                            )
                            nc.tensor.matmul(
                                pg[:, 1, :], k2t[p][64:128, ks], q2t[p][64:128, qs_],
                                start=True, stop=True, tile_position=(64, 0),
                            )
                            nc.scalar.activation(attnT[:], pg[:], AF.Exp, scale=SCALE)
                            # AV accumulation (denominator rides as ones column)
                            nc.tensor.matmul(
                                pav_a[0:65, :], v_t[kt][:, 2 * p, :], attnT[:, 0, :],
                                start=(kt == 0), stop=(kt == KT_N - 1),
                            )
                            nc.tensor.matmul(
                                pav_b[0:65, :], v_t[kt][:, 2 * p + 1, :], attnT[:, 1, :],
                                start=(kt == 0), stop=(kt == KT_N - 1),
                            )
                    with nc.named_scope(f"norm_p{p}_qb{qb}"):
                        stk = stkp.tile([128, 512], BF16, name="stk")
                        for j, pav in enumerate((pav_a, pav_b)):
                            sums = normp.tile([128, 512], mybir.dt.float32r, name="sums")
                            nc.vector.tensor_copy(sums[64:65, :], pav[64:65, :])
                            pbc = ps_misc.tile([128, 512], FP32, tag="misc", name="pbc")
                            nc.tensor.matmul(
                                pbc[0:64, :], ones1[64:65, :].bitcast(mybir.dt.float32r), sums[64:65, :],
                                start=True, stop=True, tile_position=(64, 0),
                            )
                            recipb = normp.tile([128, 512], FP32, name="recipb")
                            nc.vector.reciprocal_approx_fast(out=recipb[0:64, :], in_=pbc[0:64, :])
                            if j == 0:
                                nc.vector.tensor_tensor(out=stk[0:64, :], in0=pav[0:64, :], in1=recipb[0:64, :], op=MUL)
                            else:
                                tmpb = stkp.tile([128, 512], BF16, name="tmpb")
                                nc.vector.tensor_tensor(out=tmpb[0:64, :], in0=pav[0:64, :], in1=recipb[0:64, :], op=MUL)
                                nc.sync.dma_start(out=stk[64:128, :], in_=tmpb[0:64, :])
                        stacked.append(stk)
                with nc.named_scope(f"oproj_qb{qb}"):
                    for qs in range(4):
                        po0 = ps_misc.tile([128, 512], FP32, tag="misc", name="po0")
                        po1 = ps_misc.tile([128, 512], FP32, tag="misc", name="po1")
                        ostg = outp.tile([128, 1024], FP32, name="ostg")
                        for dt, po in enumerate((po0, po1)):
                            for p in range(2):
                                nc.tensor.matmul(
                                    po[:],
                                    stacked[p][:, qs * 128:(qs + 1) * 128],
                                    wo[:, p, dt * 512:(dt + 1) * 512],
                                    start=(p == 0), stop=(p == 1),
                                )
                            nc.vector.tensor_copy(ostg[:, dt * 512:(dt + 1) * 512], po[:])
                        nc.sync.dma_start(out=out_d[qb * 4 + qs, :, :], in_=ostg[:])

            stack.__exit__(None, None, None)

    nc.compile()
    return nc


def _host_prep(hidden_states, w_qkv):
    """Per-core input maps (host-side shard + layout prep)."""
    invf = 1.0 / (THETA ** (np.arange(0, DH, 2, dtype=np.float32) / DH))
    t = np.arange(N, dtype=np.float32)
    d_idx = np.arange(128)
    f = invf[(d_idx % 64) // 2]
    ang = t[None, :] * f[:, None]
    cos2 = np.ascontiguousarray(np.cos(ang)).astype(BF)
    sign = np.where(d_idx % 2 == 0, -1.0, 1.0).astype(np.float32)
    sin2n = np.ascontiguousarray(np.sin(ang) * sign[:, None]).astype(BF)

    xT_b = [np.ascontiguousarray(hidden_states[b].T).astype(BF).reshape(D // 128, 128, N)
            for b in range(B)]

    in_maps = []
    for c in range(8):
        b, g = c // 4, c % 4
        heads = [4 * g, 4 * g + 1, 4 * g + 2, 4 * g + 3]
        cols = []
        for off in (0, 1024):
            for h in heads:
                cols.append(w_qkv[:, off + h * 64: off + (h + 1) * 64])
        wqk = np.concatenate(cols, axis=1).astype(BF).reshape(D // 128, 128, 512)
        wv = np.concatenate([w_qkv[:, 2048 + h * 64: 2048 + (h + 1) * 64] for h in heads],
                            axis=1).astype(BF).reshape(D // 128, 128, 256)
        in_maps.append({
            "xT": xT_b[b],
            "wqk": np.ascontiguousarray(wqk),
            "wv": np.ascontiguousarray(wv),
            "cos2": cos2,
            "sin2n": sin2n,
        })
    return in_maps


def kernel(hidden_states, w_qkv, w_out, b_out, _trace=False, _tmpdir=None):
    hidden_states = np.asarray(hidden_states, dtype=np.float32)
    w_qkv = np.asarray(w_qkv, dtype=np.float32)
    w_out = np.asarray(w_out, dtype=np.float32)
    b_out = np.asarray(b_out, dtype=np.float32)

    from concourse.bass_utils import run_bass_kernel_spmd

    if "nc" not in _CACHE:
        _CACHE["nc"] = _build()
    nc = _CACHE["nc"]

    in_maps = _host_prep(hidden_states, w_qkv)
    for c in range(8):
        g = c % 4
        wo = np.ascontiguousarray(
            w_out[4 * g * 64: 4 * g * 64 + 256, :].astype(BF).reshape(2, 128, 1024))
        in_maps[c]["wo"] = wo

    kwargs = {}
    if _trace:
        kwargs = dict(trace=True, tmpdir=_tmpdir)
    res = run_bass_kernel_spmd(nc, in_maps, core_ids=list(range(8)), **kwargs)

    out = np.zeros((B, N, D), dtype=np.float32)
    for c in range(8):
        out[c // 4] += res.results[c]["out"].reshape(N, D)
    out += b_out[None, None, :]
    if _trace:
        _CACHE["last_res"] = res
    return out


# revision 11
# speedup vs baseline: 1.0099x; 1.0099x over previous
"""Trainium2 Bass kernel for nn_Attention_54245436948569.

Full multi-head attention (qkv proj + interleaved RoPE + softmax attention +
out proj) for B=2, N=2048, D=1024, H=16, DH=64, sharded over 8 NeuronCores as
(batch x head-group): core c handles batch c//4 and heads [4*(c%4), 4*(c%4)+4).

Per-core kernel computes a row-parallel partial of the out-projection
([2048, 1024] fp32); the host sums the 4 partials per batch and adds b_out
(the unshard step for row-parallel tensor parallelism).

Matmuls run in bf16 (fp32 PSUM accumulation); softmax runs in fp32 on the
scalar engine with the 1/sqrt(DH) scale folded into exp. The softmax
denominator rides the AV matmul as a ones-column appended to V.
"""

import os
import numpy as np
import ml_dtypes

B, N, D = 2, 2048, 1024
H, DH = 16, 64
THETA = 10000.0

BF = ml_dtypes.bfloat16

_CACHE = {}


def _build():
    import concourse.bass as bass
    import concourse.mybir as mybir
    import concourse.tile as tile
    from concourse import bacc

    FP32 = mybir.dt.float32
    BF16 = mybir.dt.bfloat16
    AF = mybir.ActivationFunctionType
    MUL = mybir.AluOpType.mult
    ADD = mybir.AluOpType.add

    from concourse.compiler_utils import get_compiler_flags, set_compiler_flags
    set_compiler_flags([f.replace("--enable-ldw-opt=false", "--enable-ldw-opt=true")
                        for f in get_compiler_flags()])

    nc = bacc.Bacc(None, target_bir_lowering=False)

    NT = N // 512            # 4 token 512-blocks
    KT_D = D // 128          # 8 contraction tiles for qkv
    KT_N = N // 128          # 16 k-token tiles for attention

    with tile.TileContext(nc) as tc:
        with tc.tile_pool(name="dram", bufs=1, space="DRAM") as dram:
            xT_d = dram.tile([KT_D, 128, N], BF16, kind="ExternalInput", name="xT", uniquify=False)
            wqk_d = dram.tile([KT_D, 128, 512], BF16, kind="ExternalInput", name="wqk", uniquify=False)
            wv_d = dram.tile([KT_D, 128, 256], BF16, kind="ExternalInput", name="wv", uniquify=False)
            wo_d = dram.tile([2, 128, 1024], BF16, kind="ExternalInput", name="wo", uniquify=False)
            cos_d = dram.tile([128, N], BF16, kind="ExternalInput", name="cos2", uniquify=False)
            sin_d = dram.tile([128, N], BF16, kind="ExternalInput", name="sin2n", uniquify=False)
            out_d = dram.tile([KT_N, 128, D], FP32, kind="ExternalOutput", name="out", uniquify=False)

            from contextlib import ExitStack
            ctx = stack = ExitStack()
            stack.__enter__()
            const = ctx.enter_context(tc.tile_pool(name="const", bufs=1))
            ropep = ctx.enter_context(tc.tile_pool(name="ropep", bufs=3))
            attnp = ctx.enter_context(tc.tile_pool(name="attnp", bufs=8))
            stkp = ctx.enter_context(tc.tile_pool(name="stkp", bufs=3))
            normp = ctx.enter_context(tc.tile_pool(name="normp", bufs=3))
            outp = ctx.enter_context(tc.tile_pool(name="outp", bufs=3))
            # PSUM budget (8 banks): misc 2 x 1 bank, scores 2 x 2 banks, av 2 x 1 bank
            ps_misc = ctx.enter_context(tc.tile_pool(name="ps_misc", bufs=2, space="PSUM"))
            ps_sc = ctx.enter_context(tc.tile_pool(name="ps_sc", bufs=2, space="PSUM"))
            ps_av = ctx.enter_context(tc.tile_pool(name="ps_av", bufs=2, space="PSUM"))

            # ---- persistent SBUF tensors ----
            xT = const.tile([128, KT_D, N], BF16)
            wqk = const.tile([128, KT_D, 512], BF16)
            wv = const.tile([128, KT_D, 256], BF16)
            wo = const.tile([128, 2, 1024], BF16)
            cos2 = const.tile([128, N], BF16)
            sin2n = const.tile([128, N], BF16)
            q2t = [const.tile([128, N], BF16, name=f"q2_{p}") for p in range(2)]
            k2t = [const.tile([128, N], BF16, name=f"k2_{p}") for p in range(2)]
            v_t = [const.tile([128, 4, 65], BF16, name=f"v_{tt}") for tt in range(KT_N)]
            ones1 = const.tile([128, 64], FP32)          # K=1 broadcast weights (row 64 used)

            with nc.named_scope("load"):
                nc.sync.dma_start(out=xT[:, 0:4, :], in_=xT_d[0:4].rearrange("k p n -> p k n"))
                nc.gpsimd.dma_start(out=xT[:, 4:8, :], in_=xT_d[4:8].rearrange("k p n -> p k n"))
                nc.scalar.dma_start(out=wqk[:], in_=wqk_d.rearrange("k p n -> p k n"))
                nc.scalar.dma_start(out=cos2[:], in_=cos_d[:])
                nc.scalar.dma_start(out=sin2n[:], in_=sin_d[:])
                nc.gpsimd.dma_start(out=wv[:], in_=wv_d.rearrange("k p n -> p k n"))
                nc.scalar.dma_start(out=wo[:], in_=wo_d.rearrange("k p n -> p k n"))
                nc.vector.memset(ones1[:], 1.0)
                for tt in range(KT_N):
                    nc.vector.memset(v_t[tt][:, :, 64:65], 1.0)

            pair_mask = []
            for i in range(16):
                pair_mask += [2 * i + 1, 2 * i]

            # ---- Phase 1: qkv projection + RoPE ----
            def qk_proj(m):
                dest = (q2t if m < 2 else k2t)[m % 2]
                for nt in range(NT):
                    pqk = ps_misc.tile([128, 512], FP32, tag="misc", name="pqk")
                    for kt in range(KT_D):
                        nc.tensor.matmul(
                            pqk[:],
                            wqk[:, kt, m * 128:(m + 1) * 128],
                            xT[:, kt, nt * 512:(nt + 1) * 512],
                            start=(kt == 0), stop=(kt == KT_D - 1),
                        )
                    # evict on ScalarE (frees PSUM with a single reader), then
                    # RoPE on DVE in bf16: rot = q*cos2 + pairswap(q)*sin2n
                    ts = slice(nt * 512, (nt + 1) * 512)
                    qraw = ropep.tile([128, 512], BF16, name="qraw")
                    if m >= 2:
                        nc.scalar.activation(qraw[:], pqk[:], AF.Copy)
                    else:
                        nc.vector.tensor_copy(qraw[:], pqk[:])
                    qcos = ropep.tile([128, 512], BF16, name="qcos")
                    qsw = ropep.tile([128, 512], BF16, name="qsw")
                    tmp = ropep.tile([128, 512], BF16, name="tmp")
                    nc.vector.tensor_tensor(out=qcos[:], in0=qraw[:], in1=cos2[:, ts], op=MUL)
                    nc.vector.stream_shuffle(qsw[:], qraw[:], pair_mask)
                    nc.vector.tensor_tensor(out=tmp[:], in0=qsw[:], in1=sin2n[:, ts], op=MUL)
                    nc.vector.tensor_tensor(out=dest[:, ts], in0=qcos[:], in1=tmp[:], op=ADD)

            def v_proj(tt):
                pv = ps_misc.tile([128, 512], FP32, tag="misc", name="pv")
                for kt in range(KT_D):
                    nc.tensor.matmul(
                        pv[:, 0:256],
                        xT[:, kt, tt * 128:(tt + 1) * 128],
                        wv[:, kt, :],
                        start=(kt == 0), stop=(kt == KT_D - 1),
                    )
                # scatter 4 head-slices into v_t[tt][:, h, 0:64]
                nc.vector.tensor_copy(v_t[tt][:, :, 0:64], pv[:, 0:256].rearrange("p (h d) -> p h d", d=64))

            with nc.named_scope("qkv"):
                # k-pair0 + q-pair0 first, then v (which gates AV), then pair1:
                # attention on pair0 starts while pair1 qkv streams in behind it.
                qk_proj(2)
                qk_proj(0)
                for tt in range(KT_N):
                    v_proj(tt)
                qk_proj(3)
                qk_proj(1)

            # ---- Phase 2: attention + out-proj per q-block ----
            SCALE = 1.0 / float(np.sqrt(DH))
            for qb in range(NT):
                qs_ = slice(qb * 512, (qb + 1) * 512)
                stacked = []
                for p in range(2):
                    with nc.named_scope(f"scores_p{p}_qb{qb}"):
                        pav_a = ps_av.tile([128, 512], FP32, tag="pav", name="pav_a")
                        pav_b = ps_av.tile([128, 512], FP32, tag="pav", name="pav_b")
                        for kt in range(KT_N):
                            pg = ps_sc.tile([128, 2, 512], FP32, tag="pg", name="pg")
                            attnT = attnp.tile([128, 2, 512], BF16, tag="attnT", name="attnT")
                            ks = slice(kt * 128, (kt + 1) * 128)
                            nc.tensor.matmul(
                                pg[:, 0, :], k2t[p][0:64, ks], q2t[p][0:64, qs_],
                                start=True, stop=True, tile_position=(0, 0),
                            )
                            nc.tensor.matmul(
                                pg[:, 1, :], k2t[p][64:128, ks], q2t[p][64:128, qs_],
                                start=True, stop=True, tile_position=(64, 0),
                            )
                            nc.scalar.activation(attnT[:], pg[:], AF.Exp, scale=SCALE)
                            # AV accumulation (denominator rides as ones column)
                            nc.tensor.matmul(
                                pav_a[0:65, :], v_t[kt][:, 2 * p, :], attnT[:, 0, :],
                                start=(kt == 0), stop=(kt == KT_N - 1),
                            )
                            nc.tensor.matmul(
                                pav_b[0:65, :], v_t[kt][:, 2 * p + 1, :], attnT[:, 1, :],
                                start=(kt == 0), stop=(kt == KT_N - 1),
                            )
                    with nc.named_scope(f"norm_p{p}_qb{qb}"):
                        stk = stkp.tile([128, 512], BF16, name="stk")
                        for j, pav in enumerate((pav_a, pav_b)):
                            sums = normp.tile([128, 512], mybir.dt.float32r, name="sums")
                            nc.vector.tensor_copy(sums[64:65, :], pav[64:65, :])
                            pbc = ps_misc.tile([128, 512], FP32, tag="misc", name="pbc")
                            nc.tensor.matmul(
                                pbc[0:64, :], ones1[64:65, :].bitcast(mybir.dt.float32r), sums[64:65, :],
                                start=True, stop=True, tile_position=(64, 0),
                            )
                            recipb = normp.tile([128, 512], FP32, name="recipb")
                            nc.vector.reciprocal_approx_fast(out=recipb[0:64, :], in_=pbc[0:64, :])
                            if j == 0:
                                nc.vector.tensor_tensor(out=stk[0:64, :], in0=pav[0:64, :], in1=recipb[0:64, :], op=MUL)
                            else:
                                tmpb = stkp.tile([128, 512], BF16, name="tmpb")
                                nc.vector.tensor_tensor(out=tmpb[0:64, :], in0=pav[0:64, :], in1=recipb[0:64, :], op=MUL)
                                nc.sync.dma_start(out=stk[64:128, :], in_=tmpb[0:64, :])
                        stacked.append(stk)
                with nc.named_scope(f"oproj_qb{qb}"):
                    for qs in range(4):
                        po0 = ps_misc.tile([128, 512], FP32, tag="misc", name="po0")
                        po1 = ps_misc.tile([128, 512], FP32, tag="misc", name="po1")
                        ostg = outp.tile([128, 1024], FP32, name="ostg")
                        for dt, po in enumerate((po0, po1)):
                            for p in range(2):
                                nc.tensor.matmul(
                                    po[:],
                                    stacked[p][:, qs * 128:(qs + 1) * 128],
                                    wo[:, p, dt * 512:(dt + 1) * 512],
                                    start=(p == 0), stop=(p == 1),
                                )
                            nc.vector.tensor_copy(ostg[:, dt * 512:(dt + 1) * 512], po[:])
                        nc.sync.dma_start(out=out_d[qb * 4 + qs, :, :], in_=ostg[:])

            stack.__exit__(None, None, None)

    nc.compile()
    return nc


def _host_prep(hidden_states, w_qkv):
    """Per-core input maps (host-side shard + layout prep)."""
    invf = 1.0 / (THETA ** (np.arange(0, DH, 2, dtype=np.float32) / DH))
    t = np.arange(N, dtype=np.float32)
    d_idx = np.arange(128)
    f = invf[(d_idx % 64) // 2]
    ang = t[None, :] * f[:, None]
    cos2 = np.ascontiguousarray(np.cos(ang)).astype(BF)
    sign = np.where(d_idx % 2 == 0, -1.0, 1.0).astype(np.float32)
    sin2n = np.ascontiguousarray(np.sin(ang) * sign[:, None]).astype(BF)

    xT_b = [np.ascontiguousarray(hidden_states[b].T).astype(BF).reshape(D // 128, 128, N)
            for b in range(B)]

    in_maps = []
    for c in range(8):
        b, g = c // 4, c % 4
        heads = [4 * g, 4 * g + 1, 4 * g + 2, 4 * g + 3]
        cols = []
        for off in (0, 1024):
            for h in heads:
                cols.append(w_qkv[:, off + h * 64: off + (h + 1) * 64])
        wqk = np.concatenate(cols, axis=1).astype(BF).reshape(D // 128, 128, 512)
        wv = np.concatenate([w_qkv[:, 2048 + h * 64: 2048 + (h + 1) * 64] for h in heads],
                            axis=1).astype(BF).reshape(D // 128, 128, 256)
        in_maps.append({
            "xT": xT_b[b],
            "wqk": np.ascontiguousarray(wqk),
            "wv": np.ascontiguousarray(wv),
            "cos2": cos2,
            "sin2n": sin2n,
        })
    return in_maps


def kernel(hidden_states, w_qkv, w_out, b_out, _trace=False, _tmpdir=None):
    hidden_states = np.asarray(hidden_states, dtype=np.float32)
    w_qkv = np.asarray(w_qkv, dtype=np.float32)
    w_out = np.asarray(w_out, dtype=np.float32)
    b_out = np.asarray(b_out, dtype=np.float32)

    from concourse.bass_utils import run_bass_kernel_spmd

    if "nc" not in _CACHE:
        _CACHE["nc"] = _build()
    nc = _CACHE["nc"]

    in_maps = _host_prep(hidden_states, w_qkv)
    for c in range(8):
        g = c % 4
        wo = np.ascontiguousarray(
            w_out[4 * g * 64: 4 * g * 64 + 256, :].astype(BF).reshape(2, 128, 1024))
        in_maps[c]["wo"] = wo

    kwargs = {}
    if _trace:
        kwargs = dict(trace=True, tmpdir=_tmpdir)
    res = run_bass_kernel_spmd(nc, in_maps, core_ids=list(range(8)), **kwargs)

    out = np.zeros((B, N, D), dtype=np.float32)
    for c in range(8):
        out[c // 4] += res.results[c]["out"].reshape(N, D)
    out += b_out[None, None, :]
    if _trace:
        _CACHE["last_res"] = res
    return out
